# revision 61
# baseline (speedup 1.0000x reference)
"""Trainium2 Bass kernel for nn_Decoder_4286377361994 (social-GAN style decoder).

Sharding: data-parallel over the 16 scene groups -> 2 groups (64 peds) per
core across 8 cores. All weights replicated. The recurrent 12-step scan runs
fully unrolled on-device; the scalar loss is recomputed on host from the
pred_traj output (loss = sum_t mean((rel_pos_t - gt_t)^2) depends only on
kernel outputs + inputs).

Device layout: activations are kept transposed [features-on-partitions,
peds-on-free]. Key pool-net restructuring:
  emb @ W1e.T == rel @ (W1e@Wp).T      (fold the 2->64 embedding)
  x1[g,i,j,:] = relu(u[g,j,:] - q[g,i,:])   with
      u = h_g @ W1h.T + pos_g @ Wc.T + (b1 + W1e@bp),  q = pos_g @ Wc.T
  -> x1.T is produced by ONE constant 0/1 "selection" matmul S on the PE
     (out[f,(i,j)] = SQ[j,f] + SQ[32+i,f]) instead of 1024 vector ops.
  max_j relu(x1 @ W2.T + b2) == relu(max_j(x1 @ W2.T) + b2)
  -> bias+relu deferred past the max-pool (32x less work).
"""

import os
import sys

import numpy as np

if "/opt/trn_rl_repo" not in sys.path:
    sys.path.insert(0, "/opt/trn_rl_repo")

T, B, NCORES, PED, G = 12, 512, 8, 64, 32
H, E, PRE, BOTTLE, MLP_D = 128, 64, 512, 1024, 1024

USE_F32R = os.environ.get("DEC_F32R", "1") == "1"
N_STEPS = int(os.environ.get("DEC_STEPS", str(T)))

_CACHE = {}
LAST_EXEC_NS = None
LAST_RESULTS = None


def _build(n_steps=T, use_f32r=USE_F32R):
    import concourse.bacc as bacc
    import concourse.bass as bass
    import concourse.tile as tile
    from concourse import mybir

    f32 = mybir.dt.float32
    f32r = mybir.dt.float32r
    AF = mybir.ActivationFunctionType
    AX = mybir.AxisListType

    # f32r: reduced-precision fp32 matmul mode (1 cycle/row vs 4 for fp32 at
    # N>=256, and single-instruction vs the fp32 LOW/HIGH 2-pass). Walrus
    # requires every producer of f32r-matmul operands to emit f32r-typed
    # output, so those tiles/DRAM tensors are declared f32r (same bits as
    # fp32 on the numpy side).
    rdt = f32r if use_f32r else f32

    nc = bacc.Bacc(None, target_bir_lowering=False)

    def din(name, shape, dt=f32):
        return nc.dram_tensor(name, list(shape), dt, kind="ExternalInput")

    d_WseT = din("WseT", (2, E), rdt)
    d_WihT = din("WihT", (E + 1, 4 * H), rdt)
    d_WhhT = din("WhhT", (H, 4 * H), rdt)
    d_WposT = din("WposT", (H, 2), rdt)
    d_bposT = din("bposT", (2, 1))
    d_W1hT = din("W1hT", (H, PRE), rdt)
    d_WcTb = din("WcTb", (3, PRE), rdt)
    d_negWcT = din("negWcT", (2, PRE), rdt)
    d_S = din("S", (2 * G, G * G), rdt)
    d_W2T = din("W2T", (128, PRE // 128, BOTTLE), rdt)
    d_b2T = din("b2T", (128, BOTTLE // 128))
    d_Wm1T = din("Wm1T", (128, (H + BOTTLE) // 128, MLP_D), rdt)
    d_bm1T = din("bm1T", (128, MLP_D // 128))
    d_Wm2T = din("Wm2T", (128, MLP_D // 128, H), rdt)
    d_bm2T = din("bm2T", (H, 1))
    d_ident = din("ident", (PED, PED))
    d_ones = din("ones", (1, PED), rdt)
    d_dmyw = din("dmyw", (128, 576), mybir.dt.bfloat16)
    d_h0T = din("h0T", (H, PED), rdt)
    d_c0T = din("c0T", (H, PED))
    d_lastposT = din("lastposT", (2, PED))
    d_gtsT = din("gtsT", (2, T + 1, PED), rdt)
    d_predT = nc.dram_tensor("predT", [T, 2, PED], f32, kind="ExternalOutput")

    with tile.TileContext(nc) as tc:
        with (
            tc.tile_pool(name="w", bufs=1) as wp,
            tc.tile_pool(name="state", bufs=2) as st,
            tc.tile_pool(name="act", bufs=3) as ap,
            tc.tile_pool(name="x1", bufs=3) as x1pool,
            tc.tile_pool(name="ppb", bufs=4, space=bass.MemorySpace.PSUM) as ppb,
            tc.tile_pool(name="ppm", bufs=2, space=bass.MemorySpace.PSUM) as ppm,
            tc.tile_pool(name="pps", bufs=2, space=bass.MemorySpace.PSUM) as pps,
        ):
            def wload(dram, tag, split=None):
                t = wp.tile(list(dram.shape), dram.dtype, tag=tag)
                if split is None:
                    nc.sync.dma_start(t[:], dram[:])
                else:
                    # chunked loads so early consumers start before the
                    # whole array lands
                    for k in range(split):
                        nc.sync.dma_start(t[:, k, :], dram[:, k, :])
                return t

            # state loads FIRST: step 0's LSTM must not queue behind weights
            hT = st.tile([H, PED], rdt, tag="h")
            nc.sync.dma_start(hT[:], d_h0T[:])
            cT = st.tile([H, PED], f32, tag="c")
            nc.sync.dma_start(cT[:], d_c0T[:])
            posM = wp.tile([3, PED], f32, tag="pos")
            nc.vector.memset(posM[:], 1.0)
            nc.sync.dma_start(posM[:2, :], d_lastposT[:])
            dec = wp.tile([E + 1, PED], rdt, tag="dec")
            nc.sync.dma_start(dec[E:E + 1, :], d_ones[:])
            gts = wload(d_gtsT, "gts")
            dmyw = wload(d_dmyw, "dmyw")
            WseT = wload(d_WseT, "WseT")
            WihT = wload(d_WihT, "WihT")
            WhhT = wload(d_WhhT, "WhhT")
            WposT = wload(d_WposT, "WposT")
            bposT = wload(d_bposT, "bposT")

            W1hT = wload(d_W1hT, "W1hT")
            WcTb = wload(d_WcTb, "WcTb")
            negWcT = wload(d_negWcT, "negWcT")
            S = wload(d_S, "S")
            W2T = wload(d_W2T, "W2T", split=PRE // 128)
            b2T = wload(d_b2T, "b2T")
            ident = wload(d_ident, "ident")
            bm1T = wload(d_bm1T, "bm1T")
            bm2T = wload(d_bm2T, "bm2T")
            Wm2T = wload(d_Wm2T, "Wm2T")
            Wm1T = wload(d_Wm1T, "Wm1T", split=(H + BOTTLE) // 128)

            # Cheap bf16 keep-warm matmuls: HAM re-throttles the PE to 1.2GHz
            # after any low-activity stretch and takes ~20us of the dense pool
            # phase to recover; these fillers hold the activity monitor busy
            # through the serial LSTM/MLP phases. The read-back copy prevents
            # dead-code elimination.
            dmysb = wp.tile([PED, 1], f32, tag="dmysb")

            def keep_warm(count, n=512):
                dmy = pps.tile([PED, 512], f32, tag="small")
                for _ in range(count):
                    nc.tensor.matmul(
                        dmy[:, :n], dmyw[:, :PED], dmyw[:, PED:PED + n],
                        start=True, stop=True,
                    )
                nc.scalar.copy(dmysb[:], dmy[:, :1])

            # warm the PE clock gate while the big weight DMAs stream in,
            # so step 0's pool phase doesn't run at the cold 1.2 GHz
            keep_warm(30)

            decps = pps.tile([E, PED], f32, tag="small")
            nc.tensor.matmul(decps[:], WseT[:], gts[:, 0, :], start=True, stop=True)
            nc.scalar.copy(dec[:E, :], decps[:])

            ADD = mybir.AluOpType.add

            for t in range(n_steps):
                # --- LSTM cell (gates stacked 4x128: i,f,o,g order) ---
                gps = pps.tile([H, 4, PED], f32, tag="small")
                for g in range(4):
                    nc.tensor.matmul(
                        gps[:, g, :], WihT[:, g * H:(g + 1) * H], dec[:],
                        start=True, stop=False,
                    )
                    nc.tensor.matmul(
                        gps[:, g, :], WhhT[:, g * H:(g + 1) * H], hT[:],
                        start=False, stop=True,
                    )
                # fillers run on the PE while ACT/DVE work through the
                # sigmoid -> c -> tanh -> hmid chain
                keep_warm(12)
                sig = ap.tile([H, 3, PED], f32, tag="sig")
                nc.scalar.activation(sig[:], gps[:, 0:3, :], AF.Sigmoid)
                tg = ap.tile([H, PED], f32, tag="tg")
                nc.scalar.activation(tg[:], gps[:, 3, :], AF.Tanh)
                c1 = ap.tile([H, PED], f32, tag="c1")
                nc.vector.tensor_mul(c1[:], sig[:, 1, :], cT[:])
                c2 = ap.tile([H, PED], f32, tag="c2")
                nc.gpsimd.tensor_mul(c2[:], sig[:, 0, :], tg[:])
                cT = st.tile([H, PED], f32, tag="c")
                nc.vector.tensor_add(cT[:], c1[:], c2[:])
                tch = ap.tile([H, PED], f32, tag="tch")
                nc.scalar.activation(tch[:], cT[:], AF.Tanh)
                hmid = ap.tile([H, PED], rdt, tag="hmid")
                nc.vector.tensor_mul(hmid[:], sig[:, 2, :], tch[:])

                # --- rel_pos / curr_pos ---
                rpps = pps.tile([2, PED], f32, tag="small")
                nc.tensor.matmul(rpps[:], WposT[:], hmid[:], start=True, stop=True)
                rp = ap.tile([2, PED], f32, tag="rp")
                nc.scalar.activation(rp[:], rpps[:], AF.Identity, bias=bposT[:, 0:1])
                nc.sync.dma_start(d_predT[t], rp[:])
                if t == n_steps - 1:
                    # the final carry (h, c, pos, pool, MLP) is never read:
                    # pred_traj[t] only needs rel_pos, so the whole last-step
                    # pool+MLP phase is dead work.
                    break
                # posM += rpps + bpos in one DVE op; the biased rel_pos copy
                # for the output DMA happens off the critical path on ACT.
                nc.vector.scalar_tensor_tensor(
                    posM[:2, :], rpps[:], bposT[:, 0:1], posM[:2, :], ADD, ADD
                )
                posR = ap.tile([3, PED], rdt, tag="posR")
                nc.scalar.copy(posR[:], posM[:])

                # --- decoder embedding for step t+1 (fills the gate phase) ---
                if t + 1 < n_steps:
                    decps = pps.tile([E, PED], f32, tag="small")
                    nc.tensor.matmul(
                        decps[:], WseT[:], gts[:, t + 1, :], start=True, stop=True
                    )
                    nc.scalar.copy(dec[:E, :], decps[:])

                # --- pool net ---
                # u/q for BOTH groups in one go (M=64 output rows = all peds)
                ups = ppb.tile([PED, PRE], f32, tag="big")
                nc.tensor.matmul(ups[:], hmid[:], W1hT[:], start=True, stop=False)
                nc.tensor.matmul(ups[:], posR[:], WcTb[:], start=False, stop=True)
                qps = ppb.tile([PED, PRE], f32, tag="big")
                nc.tensor.matmul(qps[:], posR[:2, :], negWcT[:], start=True, stop=True)
                # SQ_g = [u rows; -q rows] per group, copied in 128-col chunks
                # (ACT: u half, DVE: q half) so the first x1 matmul starts
                # after one chunk pair instead of a full 512-col copy.
                SQs = []
                for grp in range(PED // G):
                    sl = slice(grp * G, (grp + 1) * G)
                    SQ = ap.tile([2 * G, PRE], rdt, tag="SQ")
                    SQs.append(SQ)
                    for mf in range(PRE // 128):
                        cs = slice(mf * 128, (mf + 1) * 128)
                        nc.scalar.copy(SQ[:G, cs], ups[sl, cs])
                        nc.vector.tensor_copy(SQ[G:, cs], qps[sl, cs])
                praw = ap.tile([128, BOTTLE // 128, PED], f32, tag="praw")
                for grp in range(PED // G):
                    SQ = SQs[grp]
                    # x1.T = relu(S.T @ SQ): [512 feats, 1024 (i,j)]
                    x1s = x1pool.tile([128, PRE // 128, G * G], rdt, tag="x1s")
                    for n in range(2):
                        for mf in range(PRE // 128):
                            if grp == 1:
                                x1ps = ppm.tile([128, 512], f32, tag="mlp")
                            else:
                                x1ps = ppb.tile([128, 512], f32, tag="big")
                            nc.tensor.matmul(
                                x1ps[:],
                                SQ[:, mf * 128:(mf + 1) * 128],
                                S[:, n * 512:(n + 1) * 512],
                                start=True, stop=True,
                            )
                            dst = x1s[:, mf, n * 512:(n + 1) * 512]
                            # n=0 relus on ACT, n=1 on DVE: the n=0 column's
                            # W2 chunks start after 4 relus on one engine
                            # while the other engine prepares n=1 in parallel
                            if n == 0:
                                nc.scalar.activation(dst, x1ps[:], AF.Relu)
                            else:
                                nc.vector.tensor_relu(dst, x1ps[:])
                    # x2.T = W2 @ x1 (K=512), fused max over neighbors j;
                    # n-outer so the first W2 chunks need only the n=0 relus
                    for n in range(2):
                        for mo in range(BOTTLE // 128):
                            if grp == 0 and mo % 2 == 1:
                                x2ps = ppm.tile([128, 512], f32, tag="mlp")
                            else:
                                x2ps = ppb.tile([128, 512], f32, tag="big")
                            for k in range(PRE // 128):
                                nc.tensor.matmul(
                                    x2ps[:],
                                    W2T[:, k, mo * 128:(mo + 1) * 128],
                                    x1s[:, k, n * 512:(n + 1) * 512],
                                    start=(k == 0), stop=(k == PRE // 128 - 1),
                                )
                            nc.vector.reduce_max(
                                praw[:, mo, grp * G + n * 16:grp * G + (n + 1) * 16],
                                x2ps[:].rearrange("p (i j) -> p i j", j=G),
                                axis=AX.X,
                            )
                # deferred bias+relu of the pool output (commutes with max);
                # split per group half so group A's relus run during group B's
                # W2 phase and only B's halves sit on the serial tail
                prel = ap.tile([128, BOTTLE // 128, PED], rdt, tag="prel")
                for grp in range(PED // G):
                    gsl = slice(grp * G, (grp + 1) * G)
                    for mo in range(BOTTLE // 128):
                        nc.scalar.activation(
                            prel[:, mo, gsl], praw[:, mo, gsl], AF.Relu,
                            bias=b2T[:, mo:mo + 1],
                        )

                # --- decoder MLP layer 1, rows layout: dh2 = dh.T.T @ Wm1T ---
                # out [64 peds, 1024 feats] accumulated over the 9 dh chunks;
                # 18 N=512 matmuls instead of 72 N=64 ones.
                nkc = (H + BOTTLE) // 128
                dh2ps = []
                for n in range(2):
                    dps = ppm.tile([PED, 512], f32, tag="mlp")
                    dh2ps.append(dps)
                    for k in range(nkc):
                        lhs = hmid[:] if k == 0 else prel[:, k - 1, :]
                        nc.tensor.matmul(
                            dps[:], lhs, Wm1T[:, k, n * 512:(n + 1) * 512],
                            start=(k == 0), stop=(k == nkc - 1),
                        )
                # per-128-slice copies so each transpose starts as soon as its
                # slice lands in SBUF (instead of after two full 512 copies)
                keep_warm(6)
                dh2r = ap.tile([PED, MLP_D], f32, tag="dh2r")
                dh2 = ap.tile([128, MLP_D // 128, PED], rdt, tag="dh2")
                for mo in range(MLP_D // 128):
                    sl = slice(mo * 128, (mo + 1) * 128)
                    psl = slice((mo % 4) * 128, (mo % 4 + 1) * 128)
                    nc.vector.tensor_copy(dh2r[:, sl], dh2ps[mo // 4][:, psl])
                    # transpose back to [feats, peds]; relu+bm1 fused into the
                    # post-transpose copy (commutes with the transpose)
                    tps = pps.tile([128, PED], f32, tag="small")
                    nc.tensor.transpose(tps[:], dh2r[:, sl], ident[:])
                    nc.scalar.activation(
                        dh2[:, mo, :], tps[:], AF.Relu, bias=bm1T[:, mo:mo + 1]
                    )
                    if mo % 2 == 1:
                        keep_warm(2)
                hps = pps.tile([H, PED], f32, tag="small")
                for k in range(MLP_D // 128):
                    nc.tensor.matmul(
                        hps[:], Wm2T[:, k, :], dh2[:, k, :],
                        start=(k == 0), stop=(k == MLP_D // 128 - 1),
                    )
                hT = st.tile([H, PED], rdt, tag="h")
                nc.scalar.activation(hT[:], hps[:], AF.Relu, bias=bm2T[:, 0:1])
                keep_warm(5)

    nc.compile()
    return nc


def _prep_host(inputs):
    f32 = np.float32

    def a(name):
        return np.ascontiguousarray(np.asarray(inputs[name]), dtype=f32)

    W_ih, W_hh = a("W_ih"), a("W_hh")
    b_ih, b_hh = a("b_ih"), a("b_hh")
    Wse, bse = a("Wse"), a("bse")
    Wpos, bpos = a("Wpos"), a("bpos")
    Wp, bp = a("Wp"), a("bp")
    W1, b1 = a("W1"), a("b1")
    W2, b2 = a("W2"), a("b2")
    Wm1, bm1 = a("Wm1"), a("bm1")
    Wm2, bm2 = a("Wm2"), a("bm2")

    c = np.ascontiguousarray
    # gate order permuted torch(i,f,g,o) -> (i,f,o,g) so the three sigmoids
    # are one contiguous ACT op; all gate biases (incl. Wih@bse from the
    # decoder embedding bias) folded into an extra ones-row of WihT.
    perm = [0, 1, 3, 2]
    WihTp = W_ih.T.reshape(E, 4, H)[:, perm, :].reshape(E, 4 * H)
    b_aug = (b_ih + b_hh + W_ih @ bse).reshape(4, H)[perm].reshape(1, 4 * H)
    shared = {
        "WseT": c(Wse.T),
        "WihT": c(np.concatenate([WihTp, b_aug], axis=0)),
        "WhhT": c(W_hh.T.reshape(H, 4, H)[:, perm, :].reshape(H, 4 * H)),
        "WposT": c(Wpos.T),
        "bposT": c(bpos.reshape(2, 1)),
        "W1hT": c(W1[:, E:].T),
        "b2T": c(b2.reshape(BOTTLE // 128, 128).T),
        "bm1T": c(bm1.reshape(MLP_D // 128, 128).T),
        "bm2T": c(bm2.reshape(H, 1)),
        "ident": np.eye(PED, dtype=f32),
        "ones": np.ones((1, PED), dtype=f32),
        "dmyw": np.ones((128, 576), dtype=__import__("ml_dtypes").bfloat16),
    }
    Wc = W1[:, :E] @ Wp  # (PRE, 2)
    b1eff = b1 + W1[:, :E] @ bp
    shared["WcTb"] = c(np.concatenate([Wc.T, b1eff[None, :]], axis=0))
    shared["negWcT"] = c(-Wc.T)
    S = np.zeros((2 * G, G * G), f32)
    for i in range(G):
        S[np.arange(G), i * G + np.arange(G)] = 1.0
        S[G + i, i * G:(i + 1) * G] = 1.0
    shared["S"] = S
    shared["W2T"] = c(W2.T.reshape(PRE // 128, 128, BOTTLE).transpose(1, 0, 2))
    shared["Wm1T"] = c(Wm1.T.reshape((H + BOTTLE) // 128, 128, MLP_D).transpose(1, 0, 2))
    shared["Wm2T"] = c(Wm2.T.reshape(MLP_D // 128, 128, H).transpose(1, 0, 2))

    h0T = a("hh")[0].T  # (H, B)
    c0T = a("ch")[0].T
    lastposT = a("last_pos").T  # (2, B)
    ptr = a("pred_traj_rel")  # (T, B, 2)
    gts = np.empty((2, T + 1, B), f32)
    gts[:, 0, :] = a("last_pos_rel").T
    gts[:, 1:, :] = ptr.transpose(2, 0, 1)

    in_maps = []
    for core in range(NCORES):
        sl = slice(core * PED, (core + 1) * PED)
        m = dict(shared)
        m["h0T"] = c(h0T[:, sl])
        m["c0T"] = c(c0T[:, sl])
        m["lastposT"] = c(lastposT[:, sl])
        m["gtsT"] = c(gts[:, :, sl])
        in_maps.append(m)
    return in_maps


def kernel(**inputs):
    global LAST_EXEC_NS, LAST_RESULTS
    from concourse.bass_utils import run_bass_kernel_spmd

    key = (N_STEPS, USE_F32R)
    if key not in _CACHE:
        _CACHE[key] = _build(N_STEPS, USE_F32R)
    nc = _CACHE[key]

    in_maps = _prep_host(inputs)
    trace = os.environ.get("DEC_TRACE", "0") == "1"
    res = run_bass_kernel_spmd(
        nc, in_maps, core_ids=list(range(NCORES)), trace=trace
    )
    LAST_EXEC_NS = res.exec_time_ns
    LAST_RESULTS = res

    pred = np.empty((T, B, 2), np.float32)
    for core in range(NCORES):
        # predT per core: (T, 2, PED)
        pred[:, core * PED:(core + 1) * PED, :] = (
            res.results[core]["predT"].transpose(0, 2, 1)
        )
    ptr = np.asarray(inputs["pred_traj_rel"], np.float32)
    diff = pred[:N_STEPS] - ptr[:N_STEPS]
    loss = np.float32(0.0)
    for t in range(N_STEPS):
        loss = loss + np.mean(diff[t] ** 2, dtype=np.float32)
    return pred, np.float32(loss)


# revision 62
# speedup vs baseline: 1.0661x; 1.0661x over previous
"""Trainium2 Bass kernel for nn_Decoder_4286377361994 (social-GAN style decoder).

Sharding: data-parallel over the 16 scene groups -> 2 groups (64 peds) per
core across 8 cores. All weights replicated. The recurrent 12-step scan runs
fully unrolled on-device; the scalar loss is recomputed on host from the
pred_traj output (loss = sum_t mean((rel_pos_t - gt_t)^2) depends only on
kernel outputs + inputs).

Device layout: activations are kept transposed [features-on-partitions,
peds-on-free]. Key pool-net restructuring:
  emb @ W1e.T == rel @ (W1e@Wp).T      (fold the 2->64 embedding)
  x1[g,i,j,:] = relu(u[g,j,:] - q[g,i,:])   with
      u = h_g @ W1h.T + pos_g @ Wc.T + (b1 + W1e@bp),  q = pos_g @ Wc.T
  -> x1.T is produced by ONE constant 0/1 "selection" matmul S on the PE
     (out[f,(i,j)] = SQ[j,f] + SQ[32+i,f]) instead of 1024 vector ops.
  max_j relu(x1 @ W2.T + b2) == relu(max_j(x1 @ W2.T) + b2)
  -> bias+relu deferred past the max-pool (32x less work).
"""

import os
import sys

import numpy as np

if "/opt/trn_rl_repo" not in sys.path:
    sys.path.insert(0, "/opt/trn_rl_repo")

T, B, NCORES, PED, G = 12, 512, 8, 64, 32
H, E, PRE, BOTTLE, MLP_D = 128, 64, 512, 1024, 1024

USE_F32R = os.environ.get("DEC_F32R", "1") == "1"
N_STEPS = int(os.environ.get("DEC_STEPS", str(T)))

_CACHE = {}
LAST_EXEC_NS = None
LAST_RESULTS = None


def _build(n_steps=T, use_f32r=USE_F32R):
    import concourse.bacc as bacc
    import concourse.bass as bass
    import concourse.tile as tile
    from concourse import mybir

    f32 = mybir.dt.float32
    f32r = mybir.dt.float32r
    AF = mybir.ActivationFunctionType
    AX = mybir.AxisListType

    # f32r: reduced-precision fp32 matmul mode (1 cycle/row vs 4 for fp32 at
    # N>=256, and single-instruction vs the fp32 LOW/HIGH 2-pass). Walrus
    # requires every producer of f32r-matmul operands to emit f32r-typed
    # output, so those tiles/DRAM tensors are declared f32r (same bits as
    # fp32 on the numpy side).
    rdt = f32r if use_f32r else f32

    nc = bacc.Bacc(None, target_bir_lowering=False)

    def din(name, shape, dt=f32):
        return nc.dram_tensor(name, list(shape), dt, kind="ExternalInput")

    d_WseT = din("WseT", (2, E), rdt)
    d_WihT = din("WihT", (E + 1, 4 * H), rdt)
    d_WhhT = din("WhhT", (H, 4 * H), rdt)
    d_WposT = din("WposT", (H, 2), rdt)
    d_bposT = din("bposT", (2, 1))
    d_W1hT = din("W1hT", (H, PRE), rdt)
    d_WcTb = din("WcTb", (3, PRE), rdt)
    d_negWcT = din("negWcT", (2, PRE), rdt)
    d_S = din("S", (2 * G, G * G), rdt)
    d_W2T = din("W2T", (128, PRE // 128, BOTTLE), rdt)
    d_b2T = din("b2T", (128, BOTTLE // 128))
    d_Wm1T = din("Wm1T", (128, (H + BOTTLE) // 128, MLP_D), rdt)
    d_bm1T = din("bm1T", (128, MLP_D // 128))
    d_Wm2T = din("Wm2T", (128, MLP_D // 128, H), rdt)
    d_bm2T = din("bm2T", (H, 1))
    d_ident = din("ident", (PED, PED))
    d_ones = din("ones", (1, PED), rdt)
    d_dmyw = din("dmyw", (128, 576), mybir.dt.bfloat16)
    d_h0T = din("h0T", (H, PED), rdt)
    d_c0T = din("c0T", (H, PED))
    d_lastposT = din("lastposT", (2, PED))
    d_gtsT = din("gtsT", (2, T + 1, PED), rdt)
    d_predT = nc.dram_tensor("predT", [T, 2, PED], f32, kind="ExternalOutput")

    with tile.TileContext(nc) as tc:
        with (
            tc.tile_pool(name="w", bufs=1) as wp,
            tc.tile_pool(name="state", bufs=2) as st,
            tc.tile_pool(name="act", bufs=3) as ap,
            tc.tile_pool(name="x1", bufs=3) as x1pool,
            tc.tile_pool(name="ppb", bufs=4, space=bass.MemorySpace.PSUM) as ppb,
            tc.tile_pool(name="ppm", bufs=2, space=bass.MemorySpace.PSUM) as ppm,
            tc.tile_pool(name="pps", bufs=2, space=bass.MemorySpace.PSUM) as pps,
        ):
            def wload(dram, tag, split=None):
                t = wp.tile(list(dram.shape), dram.dtype, tag=tag)
                if split is None:
                    nc.sync.dma_start(t[:], dram[:])
                else:
                    # chunked loads so early consumers start before the
                    # whole array lands
                    for k in range(split):
                        nc.sync.dma_start(t[:, k, :], dram[:, k, :])
                return t

            # state loads FIRST: step 0's LSTM must not queue behind weights
            hT = st.tile([H, PED], rdt, tag="h")
            nc.sync.dma_start(hT[:], d_h0T[:])
            cT = st.tile([H, PED], f32, tag="c")
            nc.sync.dma_start(cT[:], d_c0T[:])
            posM = wp.tile([3, PED], f32, tag="pos")
            nc.vector.memset(posM[:], 1.0)
            nc.sync.dma_start(posM[:2, :], d_lastposT[:])
            dec = wp.tile([E + 1, PED], rdt, tag="dec")
            nc.sync.dma_start(dec[E:E + 1, :], d_ones[:])
            gts = wload(d_gtsT, "gts")
            dmyw = wload(d_dmyw, "dmyw")
            WseT = wload(d_WseT, "WseT")
            WihT = wload(d_WihT, "WihT")
            WhhT = wload(d_WhhT, "WhhT")
            WposT = wload(d_WposT, "WposT")
            bposT = wload(d_bposT, "bposT")

            W1hT = wload(d_W1hT, "W1hT")
            WcTb = wload(d_WcTb, "WcTb")
            negWcT = wload(d_negWcT, "negWcT")
            S = wload(d_S, "S")
            W2T = wload(d_W2T, "W2T", split=PRE // 128)
            b2T = wload(d_b2T, "b2T")
            ident = wload(d_ident, "ident")
            bm1T = wload(d_bm1T, "bm1T")
            bm2T = wload(d_bm2T, "bm2T")
            Wm2T = wload(d_Wm2T, "Wm2T")
            Wm1T = wload(d_Wm1T, "Wm1T", split=(H + BOTTLE) // 128)

            # Cheap bf16 keep-warm matmuls: HAM re-throttles the PE to 1.2GHz
            # after any low-activity stretch and takes ~20us of the dense pool
            # phase to recover; these fillers hold the activity monitor busy
            # through the serial LSTM/MLP phases. The read-back copy prevents
            # dead-code elimination.
            dmysb = wp.tile([PED, 1], f32, tag="dmysb")

            def keep_warm(count, n=512):
                dmy = pps.tile([PED, 512], f32, tag="small")
                for _ in range(count):
                    nc.tensor.matmul(
                        dmy[:, :n], dmyw[:, :PED], dmyw[:, PED:PED + n],
                        start=True, stop=True,
                    )
                nc.scalar.copy(dmysb[:], dmy[:, :1])

            # warm the PE clock gate while the big weight DMAs stream in,
            # so step 0's pool phase doesn't run at the cold 1.2 GHz
            keep_warm(30)

            decps = pps.tile([E, PED], f32, tag="small")
            nc.tensor.matmul(decps[:], WseT[:], gts[:, 0, :], start=True, stop=True)
            nc.scalar.copy(dec[:E, :], decps[:])

            ADD = mybir.AluOpType.add

            for t in range(n_steps):
                # --- LSTM cell (gates stacked 4x128: i,f,o,g order) ---
                gps = pps.tile([H, 4, PED], f32, tag="small")
                for g in range(4):
                    nc.tensor.matmul(
                        gps[:, g, :], WihT[:, g * H:(g + 1) * H], dec[:],
                        start=True, stop=False,
                    )
                    nc.tensor.matmul(
                        gps[:, g, :], WhhT[:, g * H:(g + 1) * H], hT[:],
                        start=False, stop=True,
                    )
                # fillers run on the PE while ACT/DVE work through the
                # sigmoid -> c -> tanh -> hmid chain
                keep_warm(12)
                sig = ap.tile([H, 3, PED], f32, tag="sig")
                nc.scalar.activation(sig[:], gps[:, 0:3, :], AF.Sigmoid)
                tg = ap.tile([H, PED], f32, tag="tg")
                nc.scalar.activation(tg[:], gps[:, 3, :], AF.Tanh)
                c1 = ap.tile([H, PED], f32, tag="c1")
                nc.vector.tensor_mul(c1[:], sig[:, 1, :], cT[:])
                c2 = ap.tile([H, PED], f32, tag="c2")
                nc.gpsimd.tensor_mul(c2[:], sig[:, 0, :], tg[:])
                cT = st.tile([H, PED], f32, tag="c")
                nc.vector.tensor_add(cT[:], c1[:], c2[:])
                tch = ap.tile([H, PED], f32, tag="tch")
                nc.scalar.activation(tch[:], cT[:], AF.Tanh)
                hmid = ap.tile([H, PED], rdt, tag="hmid")
                nc.vector.tensor_mul(hmid[:], sig[:, 2, :], tch[:])

                # --- rel_pos / curr_pos ---
                rpps = pps.tile([2, PED], f32, tag="small")
                nc.tensor.matmul(rpps[:], WposT[:], hmid[:], start=True, stop=True)
                rp = ap.tile([2, PED], f32, tag="rp")
                nc.scalar.activation(rp[:], rpps[:], AF.Identity, bias=bposT[:, 0:1])
                nc.sync.dma_start(d_predT[t], rp[:])
                if t == n_steps - 1:
                    # the final carry (h, c, pos, pool, MLP) is never read:
                    # pred_traj[t] only needs rel_pos, so the whole last-step
                    # pool+MLP phase is dead work.
                    break
                # posM += rpps + bpos in one DVE op; the biased rel_pos copy
                # for the output DMA happens off the critical path on ACT.
                nc.vector.scalar_tensor_tensor(
                    posM[:2, :], rpps[:], bposT[:, 0:1], posM[:2, :], ADD, ADD
                )
                posR = ap.tile([3, PED], rdt, tag="posR")
                nc.scalar.copy(posR[:], posM[:])

                # --- decoder embedding for step t+1 (fills the gate phase) ---
                if t + 1 < n_steps:
                    decps = pps.tile([E, PED], f32, tag="small")
                    nc.tensor.matmul(
                        decps[:], WseT[:], gts[:, t + 1, :], start=True, stop=True
                    )
                    nc.scalar.copy(dec[:E, :], decps[:])

                # --- pool net ---
                # u/q for BOTH groups in one go (M=64 output rows = all peds)
                ups = ppb.tile([PED, PRE], f32, tag="big")
                nc.tensor.matmul(ups[:], hmid[:], W1hT[:], start=True, stop=False)
                nc.tensor.matmul(ups[:], posR[:], WcTb[:], start=False, stop=True)
                qps = ppb.tile([PED, PRE], f32, tag="big")
                nc.tensor.matmul(qps[:], posR[:2, :], negWcT[:], start=True, stop=True)
                # SQ_g = [u rows; -q rows] per group, copied in 128-col chunks
                # (ACT: u half, DVE: q half) so the first x1 matmul starts
                # after one chunk pair instead of a full 512-col copy.
                SQs = []
                for grp in range(PED // G):
                    sl = slice(grp * G, (grp + 1) * G)
                    SQ = ap.tile([2 * G, PRE], rdt, tag="SQ")
                    SQs.append(SQ)
                    for mf in range(PRE // 128):
                        cs = slice(mf * 128, (mf + 1) * 128)
                        nc.scalar.copy(SQ[:G, cs], ups[sl, cs])
                        nc.vector.tensor_copy(SQ[G:, cs], qps[sl, cs])
                praw = ap.tile([128, BOTTLE // 128, PED], f32, tag="praw")
                for grp in range(PED // G):
                    SQ = SQs[grp]
                    # x1.T = relu(S.T @ SQ): [512 feats, 1024 (i,j)]
                    x1s = x1pool.tile([128, PRE // 128, G * G], rdt, tag="x1s")
                    for n in range(2):
                        for mf in range(PRE // 128):
                            x1ps = ppb.tile([128, 512], f32, tag="big")
                            nc.tensor.matmul(
                                x1ps[:],
                                SQ[:, mf * 128:(mf + 1) * 128],
                                S[:, n * 512:(n + 1) * 512],
                                start=True, stop=True,
                            )
                            dst = x1s[:, mf, n * 512:(n + 1) * 512]
                            # n=0 relus on ACT, n=1 on DVE: the n=0 column's
                            # W2 chunks start after 4 relus on one engine
                            # while the other engine prepares n=1 in parallel
                            if n == 0:
                                nc.scalar.activation(dst, x1ps[:], AF.Relu)
                            else:
                                nc.vector.tensor_relu(dst, x1ps[:])
                    # x2.T = W2 @ x1 (K=512), fused max over neighbors j;
                    # n-outer so the first W2 chunks need only the n=0 relus
                    for n in range(2):
                        for mo in range(BOTTLE // 128):
                            if grp == 0 and mo % 3 == 2:
                                x2ps = ppm.tile([128, 512], f32, tag="mlp")
                            else:
                                x2ps = ppb.tile([128, 512], f32, tag="big")
                            for k in range(PRE // 128):
                                nc.tensor.matmul(
                                    x2ps[:],
                                    W2T[:, k, mo * 128:(mo + 1) * 128],
                                    x1s[:, k, n * 512:(n + 1) * 512],
                                    start=(k == 0), stop=(k == PRE // 128 - 1),
                                )
                            nc.vector.reduce_max(
                                praw[:, mo, grp * G + n * 16:grp * G + (n + 1) * 16],
                                x2ps[:].rearrange("p (i j) -> p i j", j=G),
                                axis=AX.X,
                            )
                # deferred bias+relu of the pool output (commutes with max);
                # split per group half so group A's relus run during group B's
                # W2 phase and only B's halves sit on the serial tail
                prel = ap.tile([128, BOTTLE // 128, PED], rdt, tag="prel")
                for grp in range(PED // G):
                    gsl = slice(grp * G, (grp + 1) * G)
                    for mo in range(BOTTLE // 128):
                        nc.scalar.activation(
                            prel[:, mo, gsl], praw[:, mo, gsl], AF.Relu,
                            bias=b2T[:, mo:mo + 1],
                        )

                # --- decoder MLP layer 1, rows layout: dh2 = dh.T.T @ Wm1T ---
                # out [64 peds, 1024 feats] accumulated over the 9 dh chunks;
                # 18 N=512 matmuls instead of 72 N=64 ones.
                nkc = (H + BOTTLE) // 128
                dh2ps = []
                for n in range(2):
                    dps = ppm.tile([PED, 512], f32, tag="mlp")
                    dh2ps.append(dps)
                    for k in range(nkc):
                        lhs = hmid[:] if k == 0 else prel[:, k - 1, :]
                        nc.tensor.matmul(
                            dps[:], lhs, Wm1T[:, k, n * 512:(n + 1) * 512],
                            start=(k == 0), stop=(k == nkc - 1),
                        )
                # per-128-slice copies so each transpose starts as soon as its
                # slice lands in SBUF (instead of after two full 512 copies)
                keep_warm(6)
                dh2r = ap.tile([PED, MLP_D], f32, tag="dh2r")
                dh2 = ap.tile([128, MLP_D // 128, PED], rdt, tag="dh2")
                for mo in range(MLP_D // 128):
                    sl = slice(mo * 128, (mo + 1) * 128)
                    psl = slice((mo % 4) * 128, (mo % 4 + 1) * 128)
                    nc.vector.tensor_copy(dh2r[:, sl], dh2ps[mo // 4][:, psl])
                    # transpose back to [feats, peds]; relu+bm1 fused into the
                    # post-transpose copy (commutes with the transpose)
                    tps = pps.tile([128, PED], f32, tag="small")
                    nc.tensor.transpose(tps[:], dh2r[:, sl], ident[:])
                    nc.scalar.activation(
                        dh2[:, mo, :], tps[:], AF.Relu, bias=bm1T[:, mo:mo + 1]
                    )
                    if mo % 2 == 1:
                        keep_warm(2)
                hps = pps.tile([H, PED], f32, tag="small")
                for k in range(MLP_D // 128):
                    nc.tensor.matmul(
                        hps[:], Wm2T[:, k, :], dh2[:, k, :],
                        start=(k == 0), stop=(k == MLP_D // 128 - 1),
                    )
                hT = st.tile([H, PED], rdt, tag="h")
                nc.scalar.activation(hT[:], hps[:], AF.Relu, bias=bm2T[:, 0:1])
                keep_warm(5)

    nc.compile()
    return nc


def _prep_host(inputs):
    f32 = np.float32

    def a(name):
        return np.ascontiguousarray(np.asarray(inputs[name]), dtype=f32)

    W_ih, W_hh = a("W_ih"), a("W_hh")
    b_ih, b_hh = a("b_ih"), a("b_hh")
    Wse, bse = a("Wse"), a("bse")
    Wpos, bpos = a("Wpos"), a("bpos")
    Wp, bp = a("Wp"), a("bp")
    W1, b1 = a("W1"), a("b1")
    W2, b2 = a("W2"), a("b2")
    Wm1, bm1 = a("Wm1"), a("bm1")
    Wm2, bm2 = a("Wm2"), a("bm2")

    c = np.ascontiguousarray
    # gate order permuted torch(i,f,g,o) -> (i,f,o,g) so the three sigmoids
    # are one contiguous ACT op; all gate biases (incl. Wih@bse from the
    # decoder embedding bias) folded into an extra ones-row of WihT.
    perm = [0, 1, 3, 2]
    WihTp = W_ih.T.reshape(E, 4, H)[:, perm, :].reshape(E, 4 * H)
    b_aug = (b_ih + b_hh + W_ih @ bse).reshape(4, H)[perm].reshape(1, 4 * H)
    shared = {
        "WseT": c(Wse.T),
        "WihT": c(np.concatenate([WihTp, b_aug], axis=0)),
        "WhhT": c(W_hh.T.reshape(H, 4, H)[:, perm, :].reshape(H, 4 * H)),
        "WposT": c(Wpos.T),
        "bposT": c(bpos.reshape(2, 1)),
        "W1hT": c(W1[:, E:].T),
        "b2T": c(b2.reshape(BOTTLE // 128, 128).T),
        "bm1T": c(bm1.reshape(MLP_D // 128, 128).T),
        "bm2T": c(bm2.reshape(H, 1)),
        "ident": np.eye(PED, dtype=f32),
        "ones": np.ones((1, PED), dtype=f32),
        "dmyw": np.ones((128, 576), dtype=__import__("ml_dtypes").bfloat16),
    }
    Wc = W1[:, :E] @ Wp  # (PRE, 2)
    b1eff = b1 + W1[:, :E] @ bp
    shared["WcTb"] = c(np.concatenate([Wc.T, b1eff[None, :]], axis=0))
    shared["negWcT"] = c(-Wc.T)
    S = np.zeros((2 * G, G * G), f32)
    for i in range(G):
        S[np.arange(G), i * G + np.arange(G)] = 1.0
        S[G + i, i * G:(i + 1) * G] = 1.0
    shared["S"] = S
    shared["W2T"] = c(W2.T.reshape(PRE // 128, 128, BOTTLE).transpose(1, 0, 2))
    shared["Wm1T"] = c(Wm1.T.reshape((H + BOTTLE) // 128, 128, MLP_D).transpose(1, 0, 2))
    shared["Wm2T"] = c(Wm2.T.reshape(MLP_D // 128, 128, H).transpose(1, 0, 2))

    h0T = a("hh")[0].T  # (H, B)
    c0T = a("ch")[0].T
    lastposT = a("last_pos").T  # (2, B)
    ptr = a("pred_traj_rel")  # (T, B, 2)
    gts = np.empty((2, T + 1, B), f32)
    gts[:, 0, :] = a("last_pos_rel").T
    gts[:, 1:, :] = ptr.transpose(2, 0, 1)

    in_maps = []
    for core in range(NCORES):
        sl = slice(core * PED, (core + 1) * PED)
        m = dict(shared)
        m["h0T"] = c(h0T[:, sl])
        m["c0T"] = c(c0T[:, sl])
        m["lastposT"] = c(lastposT[:, sl])
        m["gtsT"] = c(gts[:, :, sl])
        in_maps.append(m)
    return in_maps


def kernel(**inputs):
    global LAST_EXEC_NS, LAST_RESULTS
    from concourse.bass_utils import run_bass_kernel_spmd

    key = (N_STEPS, USE_F32R)
    if key not in _CACHE:
        _CACHE[key] = _build(N_STEPS, USE_F32R)
    nc = _CACHE[key]

    in_maps = _prep_host(inputs)
    trace = os.environ.get("DEC_TRACE", "0") == "1"
    res = run_bass_kernel_spmd(
        nc, in_maps, core_ids=list(range(NCORES)), trace=trace
    )
    LAST_EXEC_NS = res.exec_time_ns
    LAST_RESULTS = res

    pred = np.empty((T, B, 2), np.float32)
    for core in range(NCORES):
        # predT per core: (T, 2, PED)
        pred[:, core * PED:(core + 1) * PED, :] = (
            res.results[core]["predT"].transpose(0, 2, 1)
        )
    ptr = np.asarray(inputs["pred_traj_rel"], np.float32)
    diff = pred[:N_STEPS] - ptr[:N_STEPS]
    loss = np.float32(0.0)
    for t in range(N_STEPS):
        loss = loss + np.mean(diff[t] ** 2, dtype=np.float32)
    return pred, np.float32(loss)
